# revision 2
# baseline (speedup 1.0000x reference)
"""Batched GCN layer on 8 TRN2 NeuronCores — fp8(e3m4)-resident version.

Problem: out[b] = Dinv (A[b]+I) Dinv (X[b] @ W + b_vec), Dinv = diag(rowsum(A+I)^-1/2)
Shapes: B=8, N=4096, DIN=DOUT=64.  Sharding: one batch element per core.

Key idea vs the bf16 two-pass baseline (54 MB HBM traffic, ~212 us): ship
Ahat^T = (A+I)^T quantized to float8 e3m4 (4 mantissa bits). At 1 byte/elem
the whole 16 MB matrix fits in SBUF (128 KiB of the 208 KiB/partition), so A
is read from HBM exactly ONCE. The d-pass (ones^T @ stripe column sums)
consumes stripes as they stream in; the aggregation matmul then replays them
from SBUF. Total HBM traffic ~18 MB vs 54 MB.

Precision design (measured vs fp64 reference: rel err ~1.4e-2 < 2e-2 gate):
  - Host ships S*(A+I)^T with S=8, clamped to 15.5 (e3m4 max). The scale
    cancels exactly: dinv_dev = (S*dhat)^-1/2 enters twice with total power
    -1 against the single factor S on Ahat. S=8 keeps entries >= 2^-3*orig
    away from the e3m4 denormal floor (2^-6), so even a denormal-flushing PE
    only loses ~0.1% of the degree mass (simulated: rel err 1.49e-2).
  - d is the rowsum of the QUANTIZED matrix -> normalization is
    self-consistent (result = exact GCN of the perturbed graph).
  - G = dinv * (XW+b) stays bf16 (stationary); mixed bf16 x fp8 matmul is
    legal on the PE (only f32 must pair with f32), accumulate in f32 PSUM.
  - H is computed AFTER dinv by scaling XTa columns with dinv (DVE) and
    re-running the tiny K=65 matmul; this lands G directly in the
    [128-partition, block, 64] layout the aggregation needs — no transpose
    dance, no DRAM bounce, no EYE matmuls (+I folded into Ahat on host).

Per-core phases (engine; approx time at 2.4 GHz PE, 358 GB/s DMA):
  1. stream 32 e3m4 stripes [128,4096] -> resident SBUF   (DMA 16 MB, 46 us)
     d_acc[c] += ones^T @ stripe (PE chases DMA,          (PE 55 us)
     8 psum banks [65,512])
  2. dinv = 1/sqrt(d) (ACT Sqrt + DVE recip, [65,4096] broadcast for free
     from the 65-row ones matmul)
  3. XTa_s = XTa * dinv (DVE); G[j] = XTa_s[:,j-block]^T @ Wb (PE, tiny)
  4. outT[c-chunk] = sum_i G_i^T @ resident_i[:,chunk]    (PE 55 us, SBUF-fed)
  5. outT *= dinv (DVE column scale), DMA out; host transposes [64,N]->[N,64].
"""

import numpy as np

B = 8
N = 4096
D = 64
P = 128
CHUNK = 512  # psum bank = 512 f32
ASCALE = 8.0
ACLAMP = 15.5

_prog_cache = {}


def _patch_tile_drain():
    """This container's walrus cannot encode sync waits on InstDrain/InstNoOp
    with >1 wait ("Too many sync wait commands"). Split the end-of-TileContext
    global-clock waits across multiple sequencer NOPs, one proc each."""
    import concourse.tile as tile_mod
    from concourse.vector_clock import ScopedClock, VectorClock

    if getattr(tile_mod.TileContext, "_drain_patched", False):
        return

    def _drain_and_barrier(self, tick_clock, wait_clock):
        g = tick_clock.global_clock
        for p in range(64):
            try:
                tick = g.peek_next(p) - 1
            except Exception:
                break
            if tick <= 0:
                continue
            vc = VectorClock()
            vc.require_at_least(p, tick)
            nop_inst = self.nc.sync.nop(nofuse=True, hint=f"pre_drain_wait_{p}")
            wait_clock.add_sem_waits(nop_inst.ins, ScopedClock({None: vc}))
        self.nc.sync.drain()
        self.nc.all_engine_barrier()
        assert self.sems is not None
        popped = self.nc._tile_sem_poison_stack.pop()
        assert popped is self._sem_poison
        self.nc.clear_and_free_semaphores(list(self.sems.allocated().values()))
        self.nc.all_engine_barrier()

    tile_mod.TileContext._drain_and_barrier = _drain_and_barrier
    tile_mod.TileContext._drain_patched = True


def _split_multiwait(nc):
    """This container's walrus encodes at most ONE sync wait per instruction
    (and none on InstDrain) — 'Too many sync wait commands' otherwise. Tile
    emits multi-wait instructions freely, so after scheduling we peel excess
    waits onto fresh same-engine NOPs inserted immediately before the
    instruction. Per-engine streams execute in order, so an earlier wait on
    the same engine is equivalent."""
    from concourse import mybir

    cnt = 0
    for bb in nc.main_func.blocks:
        insts = bb.instructions
        out = []
        changed = False
        for ins in insts:
            si = ins.sync_info
            waits = list(si.on_wait) if si is not None else []
            limit = 0 if isinstance(ins, mybir.InstDrain) else 1
            if len(waits) > limit:
                keep = waits[-limit:] if limit else []
                for w in waits[:len(waits) - limit]:
                    cnt += 1
                    nop = mybir.InstNoOp(
                        name=f"I-wsplit-{cnt}", ins=[], outs=[])
                    nop.engine = ins.engine
                    nop.sync_info = mybir.SyncInfo(on_wait=[w], on_update=[])
                    out.append(nop)
                ins.sync_info = mybir.SyncInfo(
                    on_wait=keep, on_update=list(si.on_update))
                changed = True
            out.append(ins)
        if changed:
            bb.instructions = out
    return cnt


def build_program(n=N, reps=1, trip=None):
    """Build the per-core bass program. Returns nc.

    trip: if set, wrap the body in a hardware For_i loop with that trip
    count (used for wall-clock timing: T(trip_hi) - T(trip_lo) isolates
    device time from dispatch/transfer overhead)."""
    _patch_tile_drain()
    import concourse.bass as bass
    import concourse.tile as tile
    from concourse import mybir

    n_mb = n // P
    n_ch = (n + CHUNK - 1) // CHUNK
    assert n % P == 0 and n % CHUNK == 0

    f32 = mybir.dt.float32
    bf16 = mybir.dt.bfloat16
    fp8 = mybir.dt.float8e3

    nc = bass.Bass(target_bir_lowering=False)
    AH = nc.declare_dram_parameter("AH", [n, n], fp8, isOutput=False)
    XTa = nc.declare_dram_parameter("XTa", [D + 1, n], bf16, isOutput=False)
    Wb = nc.declare_dram_parameter("Wb", [D + 1, D], bf16, isOutput=False)
    OT = nc.declare_dram_parameter("OT", [D, n], f32, isOutput=True)

    with tile.TileContext(nc) as tc:
        with tc.tile_pool(name="const", bufs=1) as cpool:
            xta_sb = cpool.tile([D + 1, n], bf16)
            nc.sync.dma_start(xta_sb[:], XTa[:])
            wb_sb = cpool.tile([D + 1, D], bf16)
            nc.sync.dma_start(wb_sb[:], Wb[:])
            ones_sb = cpool.tile([P, D + 1], bf16)
            nc.vector.memset(ones_sb[:], 1.0)

            if trip is not None:
                with tc.For_i(0, trip, 1):
                    _one_rep(nc, tc, mybir, n, n_mb, n_ch,
                             AH, OT, xta_sb, wb_sb, ones_sb)
            else:
                for rep in range(reps):
                    _one_rep(nc, tc, mybir, n, n_mb, n_ch,
                             AH, OT, xta_sb, wb_sb, ones_sb)
    _split_multiwait(nc)
    return nc


def _one_rep(nc, tc, mybir, n, n_mb, n_ch, AH, OT, xta_sb, wb_sb, ones_sb):
    f32 = mybir.dt.float32
    bf16 = mybir.dt.bfloat16
    fp8 = mybir.dt.float8e3
    D1 = D + 1

    with tc.tile_pool(name="work", bufs=1) as wpool:
        resident = wpool.tile([P, n_mb, n], fp8)
        dsq = wpool.tile([D1, n], f32)
        dinv_rep = wpool.tile([D1, n], f32)
        xta_s = wpool.tile([D1, n], bf16)
        g_all = wpool.tile([P, n_mb, D], bf16)
        out_sb = wpool.tile([D, n], f32)

        # ---- Phase 1: stream stripes in; d-pass column sums ----
        with tc.tile_pool(name="dpsum", bufs=1, space="PSUM") as dpsum:
            d_acc = [dpsum.tile([D1, CHUNK], f32, name=f"d_acc{c}",
                                tag=f"d_acc{c}") for c in range(n_ch)]
            for i in range(n_mb):
                nc.sync.dma_start(resident[:, i, :], AH[i * P:(i + 1) * P, :])
                for c in range(n_ch):
                    nc.tensor.matmul(
                        d_acc[c][:],
                        ones_sb[:],
                        resident[:, i, c * CHUNK:(c + 1) * CHUNK],
                        start=(i == 0), stop=(i == n_mb - 1))
            # ---- Phase 2: dinv = 1/sqrt(d), broadcast across 65 partitions
            for c in range(n_ch):
                nc.scalar.activation(
                    dsq[:, c * CHUNK:(c + 1) * CHUNK], d_acc[c][:],
                    mybir.ActivationFunctionType.Sqrt)
        nc.vector.reciprocal(dinv_rep[:], dsq[:])

        # ---- Phase 3: XTa_s = XTa * dinv; G = XTa_s^T @ Wb per block ----
        nc.vector.tensor_tensor(
            xta_s[:], xta_sb[:], dinv_rep[:], mybir.AluOpType.mult)
        with tc.tile_pool(name="gpsum", bufs=4, space="PSUM") as gpsum:
            for j in range(n_mb):
                gp = gpsum.tile([P, D], f32)
                nc.tensor.matmul(
                    gp[:], xta_s[:, j * P:(j + 1) * P], wb_sb[:],
                    start=True, stop=True)
                nc.vector.tensor_copy(g_all[:, j, :], gp[:])

        # ---- Phase 4: outT accumulation from SBUF-resident stripes ----
        with tc.tile_pool(name="opsum", bufs=1, space="PSUM") as opsum:
            o_acc = [opsum.tile([D, CHUNK], f32, name=f"o_acc{c}",
                                tag=f"o_acc{c}") for c in range(n_ch)]
            for i in range(n_mb):
                for c in range(n_ch):
                    nc.tensor.matmul(
                        o_acc[c][:],
                        g_all[:, i, :],
                        resident[:, i, c * CHUNK:(c + 1) * CHUNK],
                        start=(i == 0), stop=(i == n_mb - 1))
            # ---- Phase 5: outer dinv column scale + store ----
            for c in range(n_ch):
                sl = slice(c * CHUNK, (c + 1) * CHUNK)
                nc.vector.tensor_tensor(
                    out_sb[:, sl], o_acc[c][:], dinv_rep[0:D, sl],
                    mybir.AluOpType.mult)
        nc.sync.dma_start(OT[:], out_sb[:])


def _get_program(key):
    if key not in _prog_cache:
        n, reps = key
        _prog_cache[key] = build_program(n=n, reps=reps)
    return _prog_cache[key]


def make_in_maps(X, A, W, b):
    import ml_dtypes
    n = A.shape[1]
    e3 = ml_dtypes.float8_e3m4
    bf = ml_dtypes.bfloat16
    Wb = np.concatenate(
        [W.astype(np.float32), b.astype(np.float32)[None, :]], axis=0
    ).astype(bf)
    idx = np.arange(n)
    in_maps = []
    for i in range(X.shape[0]):
        AT = np.ascontiguousarray(np.asarray(A[i]).T) * np.float32(ASCALE)
        AT[idx, idx] += np.float32(ASCALE)          # +I folded in
        np.minimum(AT, np.float32(ACLAMP), out=AT)  # e3m4 max is 15.5
        XTa = np.concatenate(
            [np.ascontiguousarray(np.asarray(X[i]).T),
             np.ones((1, n), np.float32)], axis=0).astype(bf)
        in_maps.append({"AH": AT.astype(e3), "XTa": XTa, "Wb": Wb})
    return in_maps


def kernel(X, A, W, b, reps=1):
    from concourse.bass_utils import run_bass_kernel_spmd

    X = np.asarray(X, dtype=np.float32)
    A = np.asarray(A, dtype=np.float32)
    W = np.asarray(W, dtype=np.float32)
    b = np.asarray(b, dtype=np.float32)
    n_b, n, _ = A.shape
    nc = _get_program((n, reps))
    in_maps = make_in_maps(X, A, W, b)
    res = run_bass_kernel_spmd(nc, in_maps, list(range(n_b)))
    out = np.stack([res.results[i]["OT"].T for i in range(n_b)])
    return np.ascontiguousarray(out)


# revision 11
# speedup vs baseline: 1.5522x; 1.5522x over previous
"""Batched GCN layer on 8 TRN2 NeuronCores — fp8(e3m4)-resident version.

Problem: out[b] = Dinv (A[b]+I) Dinv (X[b] @ W + b_vec), Dinv = diag(rowsum(A+I)^-1/2)
Shapes: B=8, N=4096, DIN=DOUT=64.  Sharding: one batch element per core.

Key idea vs the bf16 two-pass baseline (54 MB HBM traffic, ~212 us): ship
Ahat^T = (A+I)^T quantized to float8 e3m4 (4 mantissa bits). At 1 byte/elem
the whole 16 MB matrix fits in SBUF (128 KiB of the 208 KiB/partition), so A
is read from HBM exactly ONCE. The d-pass (ones^T @ stripe column sums)
consumes stripes as they stream in; the aggregation matmul then replays them
from SBUF. Total HBM traffic ~18 MB vs 54 MB.

Precision design (measured vs fp64 reference: rel err ~1.4e-2 < 2e-2 gate):
  - Host ships S*(A+I)^T with S=8, clamped to 15.5 (e3m4 max). The scale
    cancels exactly: dinv_dev = (S*dhat)^-1/2 enters twice with total power
    -1 against the single factor S on Ahat. S=8 keeps entries >= 2^-3*orig
    away from the e3m4 denormal floor (2^-6), so even a denormal-flushing PE
    only loses ~0.1% of the degree mass (simulated: rel err 1.49e-2).
  - d is the rowsum of the QUANTIZED matrix -> normalization is
    self-consistent (result = exact GCN of the perturbed graph).
  - G = dinv * (XW+b) stays bf16 (stationary); mixed bf16 x fp8 matmul is
    legal on the PE (only f32 must pair with f32), accumulate in f32 PSUM.
  - H is computed AFTER dinv by scaling XTa columns with dinv (DVE) and
    re-running the tiny K=65 matmul; this lands G directly in the
    [128-partition, block, 64] layout the aggregation needs — no transpose
    dance, no DRAM bounce, no EYE matmuls (+I folded into Ahat on host).

Per-core phases (engine; approx time at 2.4 GHz PE, 358 GB/s DMA):
  1. stream 32 e3m4 stripes [128,4096] -> resident SBUF   (DMA 16 MB, 46 us)
     d_acc[c] += ones^T @ stripe (PE chases DMA,          (PE 55 us)
     8 psum banks [65,512])
  2. dinv = 1/sqrt(d) (ACT Sqrt + DVE recip, [65,4096] broadcast for free
     from the 65-row ones matmul)
  3. XTa_s = XTa * dinv (DVE); G[j] = XTa_s[:,j-block]^T @ Wb (PE, tiny)
  4. outT[c-chunk] = sum_i G_i^T @ resident_i[:,chunk]    (PE 55 us, SBUF-fed)
  5. outT *= dinv (DVE column scale), DMA out; host transposes [64,N]->[N,64].
"""

import numpy as np

B = 8
N = 4096
D = 64
P = 128
CHUNK = 512  # psum bank = 512 f32
ASCALE = 8.0
ACLAMP = 15.5

_prog_cache = {}


def _patch_tile_drain():
    """This container's walrus cannot encode sync waits on InstDrain/InstNoOp
    with >1 wait ("Too many sync wait commands"). Split the end-of-TileContext
    global-clock waits across multiple sequencer NOPs, one proc each."""
    import concourse.tile as tile_mod
    from concourse.vector_clock import ScopedClock, VectorClock

    if getattr(tile_mod.TileContext, "_drain_patched", False):
        return

    def _drain_and_barrier(self, tick_clock, wait_clock):
        g = tick_clock.global_clock
        for p in range(64):
            try:
                tick = g.peek_next(p) - 1
            except Exception:
                break
            if tick <= 0:
                continue
            vc = VectorClock()
            vc.require_at_least(p, tick)
            nop_inst = self.nc.sync.nop(nofuse=True, hint=f"pre_drain_wait_{p}")
            wait_clock.add_sem_waits(nop_inst.ins, ScopedClock({None: vc}))
        self.nc.sync.drain()
        self.nc.all_engine_barrier()
        assert self.sems is not None
        popped = self.nc._tile_sem_poison_stack.pop()
        assert popped is self._sem_poison
        self.nc.clear_and_free_semaphores(list(self.sems.allocated().values()))
        self.nc.all_engine_barrier()

    tile_mod.TileContext._drain_and_barrier = _drain_and_barrier
    tile_mod.TileContext._drain_patched = True


def _split_multiwait(nc):
    """This container's walrus encodes at most ONE sync wait per instruction
    (and none on InstDrain) — 'Too many sync wait commands' otherwise. Tile
    emits multi-wait instructions freely, so after scheduling we peel excess
    waits onto fresh same-engine NOPs inserted immediately before the
    instruction. Per-engine streams execute in order, so an earlier wait on
    the same engine is equivalent."""
    from concourse import mybir

    cnt = 0
    for bb in nc.main_func.blocks:
        insts = bb.instructions
        out = []
        changed = False
        for ins in insts:
            si = ins.sync_info
            waits = list(si.on_wait) if si is not None else []
            limit = 0 if isinstance(ins, mybir.InstDrain) else 1
            if len(waits) > limit:
                keep = waits[-limit:] if limit else []
                for w in waits[:len(waits) - limit]:
                    cnt += 1
                    nop = mybir.InstNoOp(
                        name=f"I-wsplit-{cnt}", ins=[], outs=[])
                    nop.engine = ins.engine
                    nop.sync_info = mybir.SyncInfo(on_wait=[w], on_update=[])
                    out.append(nop)
                ins.sync_info = mybir.SyncInfo(
                    on_wait=keep, on_update=list(si.on_update))
                changed = True
            out.append(ins)
        if changed:
            bb.instructions = out
    return cnt


def build_program(n=N, reps=1, trip=None, dve_every=3):
    """Build the per-core bass program. Returns nc.

    trip: if set, wrap the body in a hardware For_i loop with that trip
    count (used for wall-clock timing: T(trip_hi) - T(trip_lo) isolates
    device time from dispatch/transfer overhead).
    dve_every: stripes with i % dve_every == dve_every-1 have their degree
    partial-sum computed on DVE instead of PE (0 disables), so the PE's
    d-pass keeps up with DMA stripe arrival."""
    _patch_tile_drain()
    import concourse.bass as bass
    import concourse.tile as tile
    from concourse import mybir

    n_mb = n // P
    n_ch = (n + CHUNK - 1) // CHUNK
    assert n % P == 0 and n % CHUNK == 0

    f32 = mybir.dt.float32
    bf16 = mybir.dt.bfloat16
    fp8 = mybir.dt.float8e3

    nc = bass.Bass(target_bir_lowering=False)
    AH = nc.declare_dram_parameter("AH", [n, n], fp8, isOutput=False)
    XTa = nc.declare_dram_parameter("XTa", [D + 1, n], bf16, isOutput=False)
    Wb = nc.declare_dram_parameter("Wb", [D + 1, D], bf16, isOutput=False)
    OT = nc.declare_dram_parameter("OT", [D, n], f32, isOutput=True)

    with tile.TileContext(nc) as tc:
        with tc.tile_pool(name="const", bufs=1) as cpool:
            # constants ride the Activation HWDGE queue so the SP queue is
            # free to start streaming A stripes immediately
            xta_sb = cpool.tile([D + 1, n], bf16)
            nc.scalar.dma_start(xta_sb[:], XTa[:])
            wb_sb = cpool.tile([D + 1, D], bf16)
            nc.scalar.dma_start(wb_sb[:], Wb[:])
            ones_sb = cpool.tile([P, P], bf16)
            nc.vector.memset(ones_sb[:], 1.0)

            if trip is not None:
                with tc.For_i(0, trip, 1):
                    _one_rep(nc, tc, mybir, n, n_mb, n_ch, dve_every,
                             AH, OT, xta_sb, wb_sb, ones_sb)
            else:
                for rep in range(reps):
                    _one_rep(nc, tc, mybir, n, n_mb, n_ch, dve_every,
                             AH, OT, xta_sb, wb_sb, ones_sb)
    _split_multiwait(nc)
    return nc


def _one_rep(nc, tc, mybir, n, n_mb, n_ch, dve_every,
             AH, OT, xta_sb, wb_sb, ones_sb):
    f32 = mybir.dt.float32
    bf16 = mybir.dt.bfloat16
    fp8 = mybir.dt.float8e3
    D1 = D + 1
    nh = n // 2
    n_bk = n_ch // 2   # psum banks for the packed output accumulators

    # DVE also takes the second-to-last stripe so the PE's d-pass tail after
    # the final DMA arrival is a single stripe.
    dve_set = set(i for i in range(n_mb - 2)
                  if dve_every and i % dve_every == dve_every - 1)
    if dve_every:
        dve_set.add(n_mb - 2)
    pe_stripes = [i for i in range(n_mb) if i not in dve_set]

    with tc.tile_pool(name="work", bufs=1) as wpool:
        resident = wpool.tile([P, n_mb, n], fp8)
        dinv_rep = wpool.tile([P, n], f32)
        xta_s = wpool.tile([D1, n], bf16)
        g_all = wpool.tile([P, n_mb, D], bf16)
        # rows 0:128 cols 0:n = sqrt scratch; later rows 0:64 = OT[:, :n/2],
        # rows 64:128 = OT[:, n/2:]
        out_sb = wpool.tile([P, n], f32)
        partial = (wpool.tile([P, n], bf16, name="partial", tag="partial")
                   if dve_set else None)

        # ---- Phase 1: stream stripes in; d-pass column sums.
        # PE: ones^T @ stripe into 8 psum banks (M=128 so dinv broadcasts
        # to all partitions). DVE: every dve_every-th stripe goes into a
        # bf16 elementwise partial instead; one bf16 ones-matmul folds it
        # into the psum banks at the end.
        with tc.tile_pool(name="dpsum", bufs=1, space="PSUM") as dpsum:
            d_acc = [dpsum.tile([P, CHUNK], f32, name=f"d_acc{c}",
                                tag=f"d_acc{c}") for c in range(n_ch)]
            first_dve = min(dve_set) if dve_set else None
            for i in range(n_mb):
                nc.sync.dma_start(resident[:, i, :], AH[i * P:(i + 1) * P, :])
                if i in dve_set:
                    if i == first_dve:
                        nc.vector.tensor_copy(partial[:], resident[:, i, :])
                    else:
                        nc.vector.tensor_tensor(
                            partial[:], partial[:], resident[:, i, :],
                            mybir.AluOpType.add)
                else:
                    last_pe = i == pe_stripes[-1]
                    for c in range(n_ch):
                        nc.tensor.matmul(
                            d_acc[c][:],
                            ones_sb[:],
                            resident[:, i, c * CHUNK:(c + 1) * CHUNK],
                            start=(i == pe_stripes[0]),
                            stop=(last_pe and not dve_set))
            if dve_set:
                for c in range(n_ch):
                    nc.tensor.matmul(
                        d_acc[c][:],
                        ones_sb[:],
                        partial[:, c * CHUNK:(c + 1) * CHUNK],
                        start=False, stop=True)

            # ---- Phase 2a: sqrt per chunk, in psum-bank-pair order (b,
            # b+4) so the packed output accumulator banks free up in
            # allocation order ----
            for b in range(n_bk):
                for c in (b, b + n_bk):
                    sl = slice(c * CHUNK, (c + 1) * CHUNK)
                    nc.scalar.activation(
                        out_sb[:, sl], d_acc[c][:],
                        mybir.ActivationFunctionType.Sqrt)

        # ---- Phases 2b+3+4, chunk-interleaved: for each 512-column chunk
        # c, the DVE finishes dinv and XTa_s for that chunk, the PE runs
        # the four G-block matmuls (DVE copies them out of psum), and then
        # the PE aggregates the four corresponding stripes. The PE stream
        # is [G 4c..4c+3][agg 4c..4c+3] so it never waits on more of the
        # DVE chain than one chunk. Output accumulators are packed two
        # chunks per psum bank (partitions 0:64 = chunk b, 64:128 = chunk
        # b+4) so gpsum(2) + opsum(4) fit in the 8 banks.
        with tc.tile_pool(name="gpsum", bufs=2, space="PSUM") as gpsum, \
             tc.tile_pool(name="opsum", bufs=1, space="PSUM") as opsum:
            o_pack = [opsum.tile([P, CHUNK], f32, name=f"o_pack{b}",
                                 tag=f"o_pack{b}") for b in range(n_bk)]
            bpc = CHUNK // P  # stripe blocks per chunk
            for c in range(n_ch):
                sl = slice(c * CHUNK, (c + 1) * CHUNK)
                nc.vector.reciprocal(dinv_rep[:, sl], out_sb[:, sl])
                nc.vector.tensor_tensor(
                    xta_s[0:D1, sl], xta_sb[0:D1, sl], dinv_rep[0:D1, sl],
                    mybir.AluOpType.mult)
                for j in range(c * bpc, (c + 1) * bpc):
                    gp = gpsum.tile([P, D], f32)
                    nc.tensor.matmul(
                        gp[:], xta_s[:, j * P:(j + 1) * P], wb_sb[:],
                        start=True, stop=True)
                    nc.vector.tensor_copy(g_all[:, j, :], gp[:])
                for i in range(c * bpc, (c + 1) * bpc):
                    for cc in range(n_ch):
                        b, half = cc % n_bk, cc // n_bk
                        nc.tensor.matmul(
                            o_pack[b][half * D:(half + 1) * D, :],
                            g_all[:, i, :],
                            resident[:, i, cc * CHUNK:(cc + 1) * CHUNK],
                            start=(i == 0), stop=(i == n_mb - 1))
            # ---- Phase 5: outer dinv column scale + store. Row half h of
            # out_sb holds OT columns [h*n/2, (h+1)*n/2); the two output
            # DMAs ride different queues (ACT, SP) so they overlap. ----
            for half in range(2):
                for b in range(n_bk):
                    c = half * n_bk + b
                    rows = slice(half * D, (half + 1) * D)
                    sl = slice(c * CHUNK, (c + 1) * CHUNK)
                    osl = slice(b * CHUNK, (b + 1) * CHUNK)
                    nc.vector.tensor_tensor(
                        out_sb[rows, osl], o_pack[b][rows, :],
                        dinv_rep[rows, sl], mybir.AluOpType.mult)
                if half == 0:
                    nc.scalar.dma_start(OT[:, 0:nh], out_sb[0:D, 0:nh])
        nc.sync.dma_start(OT[:, nh:], out_sb[D:P, 0:nh])


def _get_program(key):
    if key not in _prog_cache:
        n, reps = key
        _prog_cache[key] = build_program(n=n, reps=reps)
    return _prog_cache[key]


def make_in_maps(X, A, W, b):
    import ml_dtypes
    n = A.shape[1]
    e3 = ml_dtypes.float8_e3m4
    bf = ml_dtypes.bfloat16
    Wb = np.concatenate(
        [W.astype(np.float32), b.astype(np.float32)[None, :]], axis=0
    ).astype(bf)
    idx = np.arange(n)
    in_maps = []
    for i in range(X.shape[0]):
        AT = np.ascontiguousarray(np.asarray(A[i]).T) * np.float32(ASCALE)
        AT[idx, idx] += np.float32(ASCALE)          # +I folded in
        np.minimum(AT, np.float32(ACLAMP), out=AT)  # e3m4 max is 15.5
        XTa = np.concatenate(
            [np.ascontiguousarray(np.asarray(X[i]).T),
             np.ones((1, n), np.float32)], axis=0).astype(bf)
        in_maps.append({"AH": AT.astype(e3), "XTa": XTa, "Wb": Wb})
    return in_maps


def kernel(X, A, W, b, reps=1):
    from concourse.bass_utils import run_bass_kernel_spmd

    X = np.asarray(X, dtype=np.float32)
    A = np.asarray(A, dtype=np.float32)
    W = np.asarray(W, dtype=np.float32)
    b = np.asarray(b, dtype=np.float32)
    n_b, n, _ = A.shape
    nc = _get_program((n, reps))
    in_maps = make_in_maps(X, A, W, b)
    res = run_bass_kernel_spmd(nc, in_maps, list(range(n_b)))
    out = np.stack([res.results[i]["OT"].T for i in range(n_b)])
    return np.ascontiguousarray(out)
